# revision 2
# baseline (speedup 1.0000x reference)
"""PASA downsample (group softmax) Trainium2 kernel, v2.

Math (per batch image, one core per image):
  xp  = reflect_pad(x, 1)                                  [64, 130, 130]
  sig = conv3x3(xp, w, stride=2)  (+ BN inference, folded) [72, 64, 64]
  e   = exp(sig)                                           [72, 64, 64]
  Z   = sum_ch e                                           [1, 64, 64]
  out[c] = (sum_k e[g(c)*9+k] * xp[c, 2i+kh, 2j+kw]) / Z   [64, 64, 64]

v2 structure (vs v1): the host pre-pads, parity-splits and casts x into a
single bf16 "planes" tensor [128, 65, 132] with partitions (h, c):
  cols 0..65   = PA: even xp cols (odd x cols; includes both halo cols)
  cols 66..130 = PB: odd xp cols
so the device does NO plane building at all: 4 banded HWDGE loads feed
both the conv matmuls and the combine directly.  All DMAs are fused:
4 loads + 4 e_rep broadcasts + 2 stores per image (v1 had ~50), which
removes the HWDGE descriptor-generation and sequencer bottlenecks.

Conv: 9 taps x 2 halves x 4 quarters of [64]x[512] matmuls into PSUM,
exp(+BN bias) on Act -> e_sb [72, (q, h, 512)].  Z via ones-matmuls into
[128, 512] PSUM, reciprocal on DVE -> rr bf16.

Combine: per quarter-pair qp, per tap: acc += e_rep[tap] * plane_view.
DVE takes the 4B-aligned kw in {0, 1} taps (2x mode), Pool (gpsimd) takes
kw=2 (misaligned -> 1x anywhere) plus the join; norm mul by rr on DVE in
bf16, final f32 cast on the otherwise idle Act engine.
"""

import os
import numpy as np
import ml_dtypes
from contextlib import ExitStack

import concourse.bass as bass
import concourse.bacc as bacc_mod
import concourse.mybir as mybir
import concourse.tile as tile
from concourse.bass_utils import run_bass_kernel_spmd

EPS = 1e-5
G = 8
N_CORES = 8

F32 = mybir.dt.float32
BF16 = mybir.dt.bfloat16
NP_BF16 = ml_dtypes.bfloat16

DT = BF16
NP_DT = NP_BF16

PBOFF = 66   # column offset of PB inside the planes tensor
PW = 132     # planes row width (elements)

# combine tap split: DVE gets the 4B-aligned kw in {0,1} taps (2x mode)
# plus tap 8 at 1x; Pool (gpsimd, software TT on Q7, ~0.42 eff) is slow,
# so it only gets 2 taps plus the join add.
DVE_TAPS = (0, 1, 3, 4, 6, 7, 8)
POOL_TAPS = (2, 5)


def build_bass(bench_iters=0):
    nc = bacc_mod.Bacc("TRN2", target_bir_lowering=False, debug=False,
                       num_swdge_queues=2)
    pl_d = nc.dram_tensor("planes", [128, 65, PW], DT, kind="ExternalInput")
    wt_d = nc.dram_tensor("wt", [64, 9, 72], DT, kind="ExternalInput")
    bnb_d = nc.dram_tensor("bnb", [72, 1], F32, kind="ExternalInput")
    ones_d = nc.dram_tensor("ones", [72, 64], DT, kind="ExternalInput")
    out_d = nc.dram_tensor("out", [64, 64, 64], F32, kind="ExternalOutput")
    # DRAM scratch for the e reorder: addr = g*36864 + 9216*(2qp+h)
    #                                        + 1024*t + 512*qq + pix
    emid_d = nc.dram_tensor("emid", [8, 4, 9216], DT, kind="Internal")

    with ExitStack() as ctx:
        tc = ctx.enter_context(tile.TileContext(nc))
        const = ctx.enter_context(tc.tile_pool(name="const", bufs=1))
        big = ctx.enter_context(tc.tile_pool(name="big", bufs=1))
        prodp = ctx.enter_context(tc.tile_pool(name="prod", bufs=4))
        psig = ctx.enter_context(tc.tile_pool(name="psig", bufs=3, space="PSUM"))
        pz = ctx.enter_context(tc.tile_pool(name="pz", bufs=2, space="PSUM"))
        pwarm = ctx.enter_context(tc.tile_pool(name="pwarm", bufs=1, space="PSUM"))

        wt_sb = const.tile([128, 9, 72], DT)  # weights duplicated on both halves
        bnb_sb = const.tile([72, 1], F32)
        ones_sb = const.tile([72, 64], DT)
        wtile = const.tile([64, 512], DT)     # PE warm-up operand
        nc.sync.dma_start(out=wt_sb[0:64], in_=wt_d[:])
        nc.sync.dma_start(out=wt_sb[64:128], in_=wt_d[:])
        nc.sync.dma_start(out=bnb_sb, in_=bnb_d[:])
        nc.sync.dma_start(out=ones_sb, in_=ones_d[:])
        nc.gpsimd.memset(wtile[:], 0.5)

        planes = big.tile([128, 65, PW], DT)
        e_sb = big.tile([72, 4096], DT)          # col = 2048qp+1024h+512qq+pix
        e_rep = big.tile([128, 2, 9, 1024], DT)  # [(h,c), qp, t, 512qq+pix]
        rr = big.tile([128, 2048], DT)           # cols = 512*q + pix
        acc_d = big.tile([128, 32, 64], DT)
        acc_p = big.tile([128, 32, 64], DT)
        out_bf = big.tile([128, 32, 64], DT)
        out_sb = big.tile([128, 32, 64], F32)

        # one-time zero-init (outside the bench loop): only needed to keep
        # CoreSim's initialized-memory checker happy — semantically every
        # read is written first.  Enabled via env for sim correctness runs.
        if os.environ.get("BASS_INIT", "0") == "1":
            for t in (e_sb, e_rep, rr, acc_d, acc_p, out_bf, out_sb):
                nc.vector.memset(t[:], 0.0)

        import contextlib
        loop_cm = tc.For_i(0, bench_iters, 1) if bench_iters else contextlib.nullcontext()
        with loop_cm:
            body(nc, pl_d, out_d, emid_d, planes, e_sb, e_rep, rr,
                 acc_d, acc_p, out_bf, out_sb, wt_sb, bnb_sb, ones_sb,
                 psig, pz, prodp, pwarm, wtile)

    nc.finalize()
    return nc


def _ap(base_ap, extra_off, dims):
    return bass.AP(tensor=base_ap.tensor, offset=base_ap.offset + extra_off,
                   ap=dims)


def tap_view(planes, t9, row0, nrows, part=slice(0, 128)):
    """Unit-stride plane view for tap t9 starting at plane row row0."""
    kh, kw = divmod(t9, 3)
    c0 = (0, PBOFF, 1)[kw]
    return planes[part, row0 + kh:row0 + kh + 2 * nrows - 1:2, c0:c0 + 64]


def body(nc, pl_d, out_d, emid_d, planes, e_sb, e_rep, rr,
         acc_d, acc_p, out_bf, out_sb, wt_sb, bnb_sb, ones_sb,
         psig, pz, prodp, pwarm, wtile):
    est = e_sb.ap[0][0]    # e_sb partition stride (elements)
    rr3 = rr.rearrange("p (a b) -> p a b", a=32)

    # ---- PE p-state warm-up: dummy matmuls so the conv runs at full clock
    ps_warm = pwarm.tile([72, 512], F32)
    for _ in range(8):
        nc.tensor.matmul(ps_warm, lhsT=wtile[:, 0:72], rhs=wtile[:],
                         start=True, stop=True)

    # ---- banded loads: rows [0,17), [17,33), [33,49), [49,65) ----
    for q in range(4):
        r0, r1 = (0 if q == 0 else 16 * q + 1), 16 * q + 17
        nc.sync.dma_start(out=planes[:, r0:r1, :], in_=pl_d[:, r0:r1, :])

    for q in range(4):
        qp, qq = divmod(q, 2)
        # ---- conv (9 taps, contraction 64) + BN bias + exp ----
        for h0 in range(2):
            ps = psig.tile([72, 512], F32)
            for i, t9 in enumerate(range(9)):
                rhs = tap_view(planes, t9, 16 * q, 8,
                               part=slice(64 * h0, 64 * h0 + 64))
                nc.tensor.matmul(ps, lhsT=wt_sb[64 * h0:64 * h0 + 64, t9, :],
                                 rhs=rhs, start=(i == 0), stop=(i == 8))
            col0 = 2048 * qp + 1024 * h0 + 512 * qq
            nc.scalar.activation(
                out=e_sb[:, col0:col0 + 512], in_=ps,
                func=mybir.ActivationFunctionType.Exp,
                bias=bnb_sb, scale=1.0,
            )

        # ---- Z (replicated to 128 partitions via ones-matmul) + recip ----
        pzt = pz.tile([128, 512], F32)
        nc.tensor.matmul(pzt[0:64, :], lhsT=ones_sb,
                         rhs=e_sb[:, 2048 * qp + 512 * qq:
                                  2048 * qp + 512 * qq + 512],
                         start=True, stop=True)
        nc.tensor.matmul(pzt[64:128, :], lhsT=ones_sb,
                         rhs=e_sb[:, 2048 * qp + 1024 + 512 * qq:
                                  2048 * qp + 1024 + 512 * qq + 512],
                         start=True, stop=True)
        with nc.allow_low_precision(reason="bf16 recip feeds bf16 combine"):
            nc.vector.reciprocal(out=rr[:, 512 * q:512 * q + 512], in_=pzt)

        if qq == 0:
            continue

        # e_rep stage A: per group g, scatter its 9 channels' qp-block to
        # DRAM scratch in (h, t, qq, pix) order.  src [9 part, 2048].
        for g in range(8):
            src = e_sb[9 * g:9 * g + 9, 2048 * qp:2048 * qp + 2048]
            dst = _ap(emid_d[:], 36864 * g + 18432 * qp,
                      [[1024, 9], [9216, 2], [1, 1024]])
            eng = nc.sync if g % 2 == 0 else nc.scalar
            eng.dma_start(out=dst, in_=src)
        # e_rep stage B: DRAM -> SBUF, duplicating each group's row to its
        # 8 channel partitions (stride-0 dup dim on the DRAM src).  Split
        # into tap-chunks so the combine can start on taps 0-2 while the
        # rest still transfers.
        for t0, t1 in ((0, 3), (3, 6), (6, 9)):
            for h0 in range(2):
                src = _ap(emid_d[:], 9216 * (2 * qp + h0) + 1024 * t0,
                          [[36864, 8], [0, 8], [1, 1024 * (t1 - t0)]])
                eng = nc.sync if h0 == 0 else nc.scalar
                eng.dma_start(
                    out=e_rep[64 * h0:64 * h0 + 64, qp, t0:t1, :], in_=src)

        # ---- quarter-pair combine ----
        row0 = 32 * qp
        ad = acc_d[:, 16 * qp:16 * qp + 16, :]
        apc = acc_p[:, 16 * qp:16 * qp + 16, :]
        for eng, taps, accv in ((nc.vector, DVE_TAPS, ad),
                                (nc.gpsimd, POOL_TAPS, apc)):
            for i, t9 in enumerate(taps):
                xv = tap_view(planes, t9, row0, 16)
                ev = e_rep[:, qp, t9, :].rearrange("p (a b) -> p a b", a=16)
                if i == 0:
                    eng.tensor_mul(accv, xv, ev)
                else:
                    prod = prodp.tile([128, 16, 64], DT)
                    eng.tensor_mul(prod, xv, ev)
                    eng.tensor_add(accv, accv, prod)
        # join partial accumulators + normalize on DVE (Pool is slow),
        # cast to f32 on Act, store per (h, qp).
        obf = out_bf[:, 16 * qp:16 * qp + 16, :]
        osl = out_sb[:, 16 * qp:16 * qp + 16, :]
        nc.vector.tensor_add(ad, ad, apc)
        nc.vector.tensor_mul(obf, ad, rr3[:, 16 * qp:16 * qp + 16, :])
        nc.scalar.copy(osl, obf)

        ost = out_sb.ap[0][0]
        for h0 in range(2):
            src = _ap(out_sb[:], 64 * h0 * ost + 1024 * qp,
                      [[ost, 64], [1, 1024]])
            dst = _ap(out_d[:], (32 * h0 + 16 * qp) * 64,
                      [[4096, 64], [1, 1024]])
            nc.sync.dma_start(out=dst, in_=src)


def host_prep(conv_w, gamma, beta, running_mean, running_var):
    inv = 1.0 / np.sqrt(np.asarray(running_var, np.float64) + EPS)
    scale = (np.asarray(gamma, np.float64) * inv).astype(np.float32)  # [72]
    bias = (np.asarray(beta, np.float64)
            - np.asarray(running_mean, np.float64) * inv * np.asarray(gamma, np.float64)
            ).astype(np.float32)
    wt = np.asarray(conv_w, np.float32) * scale[:, None, None, None]  # [72,64,3,3]
    # channel order stays group-major (co = 9g + k), the natural conv_w order
    wt = np.ascontiguousarray(wt.transpose(1, 2, 3, 0).reshape(64, 9, 72))
    return {
        "wt": wt.astype(NP_DT),
        "bnb": np.ascontiguousarray(bias.reshape(72, 1)),
        "ones": np.ones((72, 64), NP_DT),
    }


def make_planes(x_img):
    """Host-side pad + parity split + cast: x [64,128,128] f32 ->
    planes [128, 65, PW] bf16 with partition p = 64*h + c:
      cols 0..64   PA[p, r, j] = xp[c, 64h+r, 2j]     (j in 0..64)
      cols 66..129 PB[p, r, j] = xp[c, 64h+r, 2j+1]   (j in 0..63)
    """
    xp = np.pad(x_img, ((0, 0), (1, 1), (1, 1)), mode="reflect")
    pl = np.zeros((128, 65, PW), NP_BF16)
    for h in range(2):
        sl = xp[:, 64 * h:64 * h + 65, :]
        pl[64 * h:64 * h + 64, :, 0:65] = sl[:, :, 0::2]
        pl[64 * h:64 * h + 64, :, PBOFF:PBOFF + 64] = sl[:, :, 1:129:2]
    return pl


_NC_CACHE = {}


def kernel(x, conv_w, gamma, beta, running_mean, running_var):
    x = np.asarray(x, np.float32)
    n = x.shape[0]
    aux = host_prep(conv_w, gamma, beta, running_mean, running_var)
    if "nc" not in _NC_CACHE:
        _NC_CACHE["nc"] = build_bass()
    nc = _NC_CACHE["nc"]
    in_maps = [dict(aux, planes=make_planes(x[i])) for i in range(n)]
    res = run_bass_kernel_spmd(nc, in_maps, core_ids=list(range(n)))
    return np.stack([r["out"] for r in res.results], axis=0)


if __name__ == "__main__":
    rng = np.random.default_rng(0)
    x = rng.standard_normal((8, 64, 128, 128), dtype=np.float32)
    cw = (rng.standard_normal((72, 64, 3, 3)) * np.sqrt(2.0 / (72 * 9))).astype(np.float32)
    out = kernel(x, cw, np.ones(72, np.float32), np.zeros(72, np.float32),
                 np.zeros(72, np.float32), np.ones(72, np.float32))
    print(out.shape, out.dtype)


# revision 3
# speedup vs baseline: 1.2007x; 1.2007x over previous
"""PASA downsample (group softmax) Trainium2 kernel, v2.

Math (per batch image, one core per image):
  xp  = reflect_pad(x, 1)                                  [64, 130, 130]
  sig = conv3x3(xp, w, stride=2)  (+ BN inference, folded) [72, 64, 64]
  e   = exp(sig)                                           [72, 64, 64]
  Z   = sum_ch e                                           [1, 64, 64]
  out[c] = (sum_k e[g(c)*9+k] * xp[c, 2i+kh, 2j+kw]) / Z   [64, 64, 64]

v2 structure (vs v1): the host pre-pads, parity-splits and casts x into a
single bf16 "planes" tensor [128, 65, 132] with partitions (h, c):
  cols 0..65   = PA: even xp cols (odd x cols; includes both halo cols)
  cols 66..130 = PB: odd xp cols
so the device does NO plane building at all: 4 banded HWDGE loads feed
both the conv matmuls and the combine directly.  All DMAs are fused:
4 loads + 4 e_rep broadcasts + 2 stores per image (v1 had ~50), which
removes the HWDGE descriptor-generation and sequencer bottlenecks.

Conv: 9 taps x 2 halves x 4 quarters of [64]x[512] matmuls into PSUM,
exp(+BN bias) on Act -> e_sb [72, (q, h, 512)].  Z via ones-matmuls into
[128, 512] PSUM, reciprocal on DVE -> rr bf16.

Combine: per quarter-pair qp, per tap: acc += e_rep[tap] * plane_view.
DVE takes the 4B-aligned kw in {0, 1} taps (2x mode), Pool (gpsimd) takes
kw=2 (misaligned -> 1x anywhere) plus the join; norm mul by rr on DVE in
bf16, final f32 cast on the otherwise idle Act engine.
"""

import os
import numpy as np
import ml_dtypes
from contextlib import ExitStack

import concourse.bass as bass
import concourse.bacc as bacc_mod
import concourse.mybir as mybir
import concourse.tile as tile
from concourse.bass_utils import run_bass_kernel_spmd

EPS = 1e-5
G = 8
N_CORES = 8

F32 = mybir.dt.float32
BF16 = mybir.dt.bfloat16
NP_BF16 = ml_dtypes.bfloat16

DT = BF16
NP_DT = NP_BF16

PBOFF = 66   # column offset of PB inside the planes tensor
PW = 132     # planes row width (elements)

# combine tap split: DVE gets the 4B-aligned kw in {0,1} taps (2x mode)
# plus tap 8 at 1x; Pool (gpsimd, software TT on Q7, ~0.42 eff) is slow,
# so it only gets 2 taps plus the join add.
DVE_TAPS = (0, 1, 3, 4, 6, 7, 8)
POOL_TAPS = (2, 5)


def build_bass(bench_iters=0):
    nc = bacc_mod.Bacc("TRN2", target_bir_lowering=False, debug=False,
                       num_swdge_queues=2)
    pl_d = nc.dram_tensor("planes", [128, 65, PW], DT, kind="ExternalInput")
    wt_d = nc.dram_tensor("wt", [64, 9, 72], DT, kind="ExternalInput")
    bnb_d = nc.dram_tensor("bnb", [72, 1], F32, kind="ExternalInput")
    ones_d = nc.dram_tensor("ones", [72, 64], DT, kind="ExternalInput")
    out_d = nc.dram_tensor("out", [64, 64, 64], F32, kind="ExternalOutput")
    # DRAM scratch for the e reorder: addr = g*36864 + 9216*(2qp+h)
    #                                        + 1024*t + 512*qq + pix
    emid_d = nc.dram_tensor("emid", [8, 4, 9216], DT, kind="Internal")

    with ExitStack() as ctx:
        tc = ctx.enter_context(tile.TileContext(nc))
        const = ctx.enter_context(tc.tile_pool(name="const", bufs=1))
        big = ctx.enter_context(tc.tile_pool(name="big", bufs=1))
        prodp = ctx.enter_context(tc.tile_pool(name="prod", bufs=4))
        psig = ctx.enter_context(tc.tile_pool(name="psig", bufs=3, space="PSUM"))
        pz = ctx.enter_context(tc.tile_pool(name="pz", bufs=2, space="PSUM"))
        pwarm = ctx.enter_context(tc.tile_pool(name="pwarm", bufs=1, space="PSUM"))

        wt_sb = const.tile([128, 9, 72], DT)  # weights duplicated on both halves
        bnb_sb = const.tile([72, 1], F32)
        ones_sb = const.tile([72, 64], DT)
        wtile = const.tile([64, 512], DT)     # PE warm-up operand
        nc.sync.dma_start(out=wt_sb[0:64], in_=wt_d[:])
        nc.sync.dma_start(out=wt_sb[64:128], in_=wt_d[:])
        nc.sync.dma_start(out=bnb_sb, in_=bnb_d[:])
        nc.sync.dma_start(out=ones_sb, in_=ones_d[:])
        nc.gpsimd.memset(wtile[:], 0.5)

        planes = big.tile([128, 65, PW], DT)
        e_sb = big.tile([72, 4096], DT)          # col = 2048qp+1024h+512qq+pix
        e_rep = big.tile([128, 2, 9, 1024], DT)  # [(h,c), qp, t, 512qq+pix]
        rr = big.tile([128, 2048], DT)           # cols = 512*q + pix
        acc_d = big.tile([128, 32, 64], DT)
        acc_p = big.tile([128, 32, 64], DT)
        out_bf = big.tile([128, 32, 64], DT)
        out_sb = big.tile([128, 32, 64], F32)

        # one-time zero-init (outside the bench loop): only needed to keep
        # CoreSim's initialized-memory checker happy — semantically every
        # read is written first.  Enabled via env for sim correctness runs.
        if os.environ.get("BASS_INIT", "0") == "1":
            for t in (e_sb, e_rep, rr, acc_d, acc_p, out_bf, out_sb):
                nc.vector.memset(t[:], 0.0)

        import contextlib
        loop_cm = tc.For_i(0, bench_iters, 1) if bench_iters else contextlib.nullcontext()
        with loop_cm:
            body(nc, pl_d, out_d, emid_d, planes, e_sb, e_rep, rr,
                 acc_d, acc_p, out_bf, out_sb, wt_sb, bnb_sb, ones_sb,
                 psig, pz, prodp, pwarm, wtile)

    nc.finalize()
    return nc


def _ap(base_ap, extra_off, dims):
    return bass.AP(tensor=base_ap.tensor, offset=base_ap.offset + extra_off,
                   ap=dims)


def tap_view(planes, t9, row0, nrows, part=slice(0, 128)):
    """Unit-stride plane view for tap t9 starting at plane row row0."""
    kh, kw = divmod(t9, 3)
    c0 = (0, PBOFF, 1)[kw]
    return planes[part, row0 + kh:row0 + kh + 2 * nrows - 1:2, c0:c0 + 64]


def body(nc, pl_d, out_d, emid_d, planes, e_sb, e_rep, rr,
         acc_d, acc_p, out_bf, out_sb, wt_sb, bnb_sb, ones_sb,
         psig, pz, prodp, pwarm, wtile):
    est = e_sb.ap[0][0]    # e_sb partition stride (elements)
    rr3 = rr.rearrange("p (a b) -> p a b", a=32)

    # ---- PE p-state warm-up: dummy matmuls so the conv runs at full clock
    ps_warm = pwarm.tile([72, 512], F32)
    for _ in range(8):
        nc.tensor.matmul(ps_warm, lhsT=wtile[:, 0:72], rhs=wtile[:],
                         start=True, stop=True)

    # ---- banded loads: rows [0,17), [17,33), [33,49), [49,65) ----
    for q in range(4):
        r0, r1 = (0 if q == 0 else 16 * q + 1), 16 * q + 17
        nc.sync.dma_start(out=planes[:, r0:r1, :], in_=pl_d[:, r0:r1, :])

    for q in range(4):
        qp, qq = divmod(q, 2)
        # ---- conv (9 taps, contraction 64) + BN bias + exp ----
        for h0 in range(2):
            ps = psig.tile([72, 512], F32)
            for i, t9 in enumerate(range(9)):
                rhs = tap_view(planes, t9, 16 * q, 8,
                               part=slice(64 * h0, 64 * h0 + 64))
                nc.tensor.matmul(ps, lhsT=wt_sb[64 * h0:64 * h0 + 64, t9, :],
                                 rhs=rhs, start=(i == 0), stop=(i == 8))
            col0 = 2048 * qp + 1024 * h0 + 512 * qq
            nc.scalar.activation(
                out=e_sb[:, col0:col0 + 512], in_=ps,
                func=mybir.ActivationFunctionType.Exp,
                bias=bnb_sb, scale=1.0,
            )

        # ---- Z (replicated to 128 partitions via ones-matmul) + recip ----
        pzt = pz.tile([128, 512], F32)
        nc.tensor.matmul(pzt[0:64, :], lhsT=ones_sb,
                         rhs=e_sb[:, 2048 * qp + 512 * qq:
                                  2048 * qp + 512 * qq + 512],
                         start=True, stop=True)
        nc.tensor.matmul(pzt[64:128, :], lhsT=ones_sb,
                         rhs=e_sb[:, 2048 * qp + 1024 + 512 * qq:
                                  2048 * qp + 1024 + 512 * qq + 512],
                         start=True, stop=True)
        with nc.allow_low_precision(reason="bf16 recip feeds bf16 combine"):
            nc.vector.reciprocal(out=rr[:, 512 * q:512 * q + 512], in_=pzt)

        if qq == 0:
            continue

        # e_rep fan-out: single-hop broadcast straight from tap-major
        # e_sb (channel row = 8t+g).  For tap t the 8 groups are the
        # consecutive partitions 8t..8t+8, so the HW-proven unit-stride
        # partition walk + stride-0 dup pattern applies directly.
        for t9 in range(9):
            for h0 in range(2):
                src = _ap(e_sb[:], 8 * t9 * est + 2048 * qp + 1024 * h0,
                          [[est, 8], [0, 8], [1, 1024]])
                eng = nc.sync if (t9 + h0) % 2 == 0 else nc.scalar
                eng.dma_start(
                    out=e_rep[64 * h0:64 * h0 + 64, qp, t9, :], in_=src)

        # ---- quarter-pair combine ----
        row0 = 32 * qp
        ad = acc_d[:, 16 * qp:16 * qp + 16, :]
        apc = acc_p[:, 16 * qp:16 * qp + 16, :]
        for eng, taps, accv in ((nc.vector, DVE_TAPS, ad),
                                (nc.gpsimd, POOL_TAPS, apc)):
            for i, t9 in enumerate(taps):
                xv = tap_view(planes, t9, row0, 16)
                ev = e_rep[:, qp, t9, :].rearrange("p (a b) -> p a b", a=16)
                if i == 0:
                    eng.tensor_mul(accv, xv, ev)
                else:
                    prod = prodp.tile([128, 16, 64], DT)
                    eng.tensor_mul(prod, xv, ev)
                    eng.tensor_add(accv, accv, prod)
        # join partial accumulators + normalize on DVE (Pool is slow),
        # cast to f32 on Act, store per (h, qp).
        obf = out_bf[:, 16 * qp:16 * qp + 16, :]
        osl = out_sb[:, 16 * qp:16 * qp + 16, :]
        nc.vector.tensor_add(ad, ad, apc)
        nc.vector.tensor_mul(obf, ad, rr3[:, 16 * qp:16 * qp + 16, :])
        nc.scalar.copy(osl, obf)

        ost = out_sb.ap[0][0]
        for h0 in range(2):
            src = _ap(out_sb[:], 64 * h0 * ost + 1024 * qp,
                      [[ost, 64], [1, 1024]])
            dst = _ap(out_d[:], (32 * h0 + 16 * qp) * 64,
                      [[4096, 64], [1, 1024]])
            nc.sync.dma_start(out=dst, in_=src)


def host_prep(conv_w, gamma, beta, running_mean, running_var):
    inv = 1.0 / np.sqrt(np.asarray(running_var, np.float64) + EPS)
    scale = (np.asarray(gamma, np.float64) * inv).astype(np.float32)  # [72]
    bias = (np.asarray(beta, np.float64)
            - np.asarray(running_mean, np.float64) * inv * np.asarray(gamma, np.float64)
            ).astype(np.float32)
    wt = np.asarray(conv_w, np.float32) * scale[:, None, None, None]  # [72,64,3,3]
    # permute output channels to tap-major order (row 8*k + g) so each
    # tap's 8 group-rows are consecutive partitions for the broadcast
    perm = np.array([g * 9 + k for k in range(9) for g in range(G)])
    wt = wt[perm]
    bias = bias[perm]
    wt = np.ascontiguousarray(wt.transpose(1, 2, 3, 0).reshape(64, 9, 72))
    return {
        "wt": wt.astype(NP_DT),
        "bnb": np.ascontiguousarray(bias.reshape(72, 1)),
        "ones": np.ones((72, 64), NP_DT),
    }


def make_planes(x_img):
    """Host-side pad + parity split + cast: x [64,128,128] f32 ->
    planes [128, 65, PW] bf16 with partition p = 64*h + c:
      cols 0..64   PA[p, r, j] = xp[c, 64h+r, 2j]     (j in 0..64)
      cols 66..129 PB[p, r, j] = xp[c, 64h+r, 2j+1]   (j in 0..63)
    """
    xp = np.pad(x_img, ((0, 0), (1, 1), (1, 1)), mode="reflect")
    pl = np.zeros((128, 65, PW), NP_BF16)
    for h in range(2):
        sl = xp[:, 64 * h:64 * h + 65, :]
        pl[64 * h:64 * h + 64, :, 0:65] = sl[:, :, 0::2]
        pl[64 * h:64 * h + 64, :, PBOFF:PBOFF + 64] = sl[:, :, 1:129:2]
    return pl


_NC_CACHE = {}


def kernel(x, conv_w, gamma, beta, running_mean, running_var):
    x = np.asarray(x, np.float32)
    n = x.shape[0]
    aux = host_prep(conv_w, gamma, beta, running_mean, running_var)
    if "nc" not in _NC_CACHE:
        _NC_CACHE["nc"] = build_bass()
    nc = _NC_CACHE["nc"]
    in_maps = [dict(aux, planes=make_planes(x[i])) for i in range(n)]
    res = run_bass_kernel_spmd(nc, in_maps, core_ids=list(range(n)))
    return np.stack([r["out"] for r in res.results], axis=0)


if __name__ == "__main__":
    rng = np.random.default_rng(0)
    x = rng.standard_normal((8, 64, 128, 128), dtype=np.float32)
    cw = (rng.standard_normal((72, 64, 3, 3)) * np.sqrt(2.0 / (72 * 9))).astype(np.float32)
    out = kernel(x, cw, np.ones(72, np.float32), np.zeros(72, np.float32),
                 np.zeros(72, np.float32), np.ones(72, np.float32))
    print(out.shape, out.dtype)
